# revision 2
# baseline (speedup 1.0000x reference)
import numpy as np
import ml_dtypes

B, H, N, D = 4, 12, 8192, 64
M = 128
NCORES = 8
PAIRS = (B * H) // NCORES
NCHUNK = 512
NCH = N // NCHUNK
NT = N // 128

_cache = {}


def _build():
    if "nc" in _cache:
        return _cache["nc"]
    import concourse.bacc as bacc
    import concourse.mybir as mybir
    import concourse.tile as tile

    f32, f32r, bf16 = mybir.dt.float32, mybir.dt.float32r, mybir.dt.bfloat16
    AF = mybir.ActivationFunctionType

    nc = bacc.Bacc("TRN2", target_bir_lowering=False, debug=False)
    QT = nc.declare_dram_parameter("QT", [PAIRS, 64, N], f32, isOutput=False)
    KT = nc.declare_dram_parameter("KT", [PAIRS, 64, N], f32, isOutput=False)
    Vb = nc.declare_dram_parameter("Vb", [PAIRS, N, 64], bf16, isOutput=False)
    NCT = nc.declare_dram_parameter("NCT", [PAIRS, 64, M], f32, isOutput=False)
    NRT = nc.declare_dram_parameter("NRT", [PAIRS, 64, M], f32, isOutput=False)
    GS = nc.declare_dram_parameter("GS", [1, 1], f32, isOutput=False)
    XO = nc.declare_dram_parameter("XO", [PAIRS, N, 64], f32, isOutput=True)

    with tile.TileContext(nc) as tc:
        with (tc.tile_pool(name="p", bufs=1) as pool,
              tc.tile_pool(name="pd", bufs=2) as poold,
              tc.tile_pool(name="ps", bufs=2, space="PSUM") as psum,
              tc.tile_pool(name="pss", bufs=1, space="PSUM") as pss):
            ident_bf = pool.tile([128, 128], bf16, tag="ident")
            nc.gpsimd.memset(ident_bf[:], 0.0)
            nc.gpsimd.affine_select(out=ident_bf[:], in_=ident_bf[:],
                compare_op=mybir.AluOpType.not_equal, fill=1.0, base=0,
                pattern=[[-1, 128]], channel_multiplier=1)
            i7 = pool.tile([128, 128], bf16, tag="i7")
            nc.gpsimd.memset(i7[:], 0.0)
            nc.gpsimd.affine_select(out=i7[:], in_=i7[:],
                compare_op=mybir.AluOpType.not_equal, fill=7.0, base=0,
                pattern=[[-1, 128]], channel_multiplier=1)
            i15 = pool.tile([128, 128], bf16, tag="i15")
            nc.gpsimd.memset(i15[:], 0.0)
            nc.gpsimd.affine_select(out=i15[:], in_=i15[:],
                compare_op=mybir.AluOpType.not_equal, fill=15.0, base=0,
                pattern=[[-1, 128]], channel_multiplier=1)
            i13 = pool.tile([128, 128], bf16, tag="i13")
            nc.gpsimd.memset(i13[:], 0.0)
            nc.gpsimd.affine_select(out=i13[:], in_=i13[:],
                compare_op=mybir.AluOpType.not_equal, fill=13.0, base=0,
                pattern=[[-1, 128]], channel_multiplier=1)
            ones_row = pool.tile([1, 128], f32, tag="ones_row")
            nc.vector.memset(ones_row[:], 1.0)
            gs_sb = pool.tile([1, 1], f32, tag="gs_sb")
            nc.sync.dma_start(gs_sb[:], GS[:])
            ps_bc = pss.tile([128, 1], f32, tag="ps_bc")
            nc.tensor.matmul(ps_bc[:], ones_row[:], gs_sb[:], start=True, stop=True)
            gsb = pool.tile([128, 1], f32, tag="gsb")
            nc.scalar.copy(gsb[:], ps_bc[:])

            for p in range(PAIRS):
                qt_r = pool.tile([64, N], f32r, tag="qt")
                kt_r = pool.tile([64, N], f32r, tag="kt")
                nc.gpsimd.dma_start(qt_r[:], QT[p])
                nc.gpsimd.dma_start(kt_r[:], KT[p])
                v_bf = pool.tile([128, NT, 64], bf16, tag="v")
                nc.sync.dma_start(v_bf[:], Vb[p].rearrange("(t pp) d -> pp t d", pp=128))
                nct_r = pool.tile([64, M], f32r, tag="nctr")
                nrt_r = pool.tile([64, M], f32r, tag="nrtr")
                nc.gpsimd.dma_start(nct_r[:], NCT[p])
                nc.gpsimd.dma_start(nrt_r[:], NRT[p])
                nct32 = pool.tile([64, M], f32, tag="nct32")
                nrt32 = pool.tile([64, M], f32, tag="nrt32")
                nc.sync.dma_start(nct32[:], NCT[p])
                nc.sync.dma_start(nrt32[:], NRT[p])

                er = pool.tile([128, N], bf16, tag="er")
                racc = pool.tile([128, NCH], f32, tag="racc")
                for j in range(NCH):
                    ps_r = psum.tile([128, NCHUNK], f32, tag="ps_big")
                    nc.tensor.matmul(ps_r[:], nrt_r[:], kt_r[:, j*NCHUNK:(j+1)*NCHUNK],
                                     start=True, stop=True)
                    nc.scalar.activation(er[:, j*NCHUNK:(j+1)*NCHUNK], ps_r[:],
                                         AF.Exp, accum_out=racc[:, j:j+1])
                ert = pool.tile([128, NT, 128], bf16, tag="ert")
                nc.sync.dma_start_transpose(ert[:], er[:])
                ps_S = pss.tile([128, 64], f32, tag="ps_s")
                for t in range(NT):
                    nc.tensor.matmul(ps_S[:], ert[:, t, :], v_bf[:, t, :],
                                     start=(t == 0), stop=(t == NT - 1))
                rsum = pool.tile([128, 1], f32, tag="rsum")
                nc.scalar.activation(racc[:], racc[:], AF.Copy, accum_out=rsum[:])
                rrec = pool.tile([128, 1], f32, tag="rrec")
                nc.vector.reciprocal(rrec[:], rsum[:])
                s_bf = pool.tile([128, 64], bf16, tag="s_bf")
                nc.vector.tensor_scalar_mul(s_bf[:], ps_S[:], rrec[:])

                ps_m = pss.tile([128, 128], f32, tag="ps_m")
                nc.tensor.matmul(ps_m[:], nrt32[:], nct32[:], start=True, stop=True)
                e_m = pool.tile([128, 128], f32, tag="e_m")
                msum = pool.tile([128, 1], f32, tag="msum")
                nc.scalar.activation(e_m[:], ps_m[:], AF.Exp, accum_out=msum[:])
                mrec = pool.tile([128, 1], f32, tag="mrec")
                nc.vector.reciprocal(mrec[:], msum[:])
                k2_bf = pool.tile([128, 128], bf16, tag="k2")
                nc.vector.tensor_scalar_mul(k2_bf[:], e_m[:], mrec[:])

                ps_t = pss.tile([128, 128], bf16, tag="ps_m")
                nc.tensor.transpose(ps_t[:], k2_bf[:], ident_bf[:])
                k2t_bf = pool.tile([128, 128], bf16, tag="k2t")
                nc.scalar.copy(k2t_bf[:], ps_t[:])
                vm_bf = poold.tile([128, 128], bf16, tag="vm")
                nc.vector.tensor_scalar_mul(vm_bf[:], ps_t[:], gsb[:])
                for it in range(6):
                    ps_kv = pss.tile([128, 128], f32, tag="ps_m")
                    nc.tensor.matmul(ps_kv[:], k2t_bf[:], vm_bf[:], start=True, stop=True)
                    kv_bf = poold.tile([128, 128], bf16, tag="kv")
                    nc.scalar.copy(kv_bf[:], ps_kv[:])
                    t1 = poold.tile([128, 128], bf16, tag="t1")
                    nc.vector.tensor_sub(t1[:], i7[:], kv_bf[:])
                    ps_kvt = pss.tile([128, 128], bf16, tag="ps_m")
                    nc.tensor.transpose(ps_kvt[:], kv_bf[:], ident_bf[:])
                    kvt_bf = poold.tile([128, 128], bf16, tag="kvt")
                    nc.scalar.copy(kvt_bf[:], ps_kvt[:])
                    ps_t2 = pss.tile([128, 128], f32, tag="ps_m")
                    nc.tensor.matmul(ps_t2[:], kvt_bf[:], t1[:], start=True, stop=True)
                    t3 = poold.tile([128, 128], bf16, tag="t3")
                    nc.vector.tensor_sub(t3[:], i15[:], ps_t2[:])
                    ps_t4 = pss.tile([128, 128], f32, tag="ps_m")
                    nc.tensor.matmul(ps_t4[:], kvt_bf[:], t3[:], start=True, stop=True)
                    t5 = poold.tile([128, 128], bf16, tag="t5")
                    nc.vector.tensor_sub(t5[:], i13[:], ps_t4[:])
                    ps_vt = pss.tile([128, 128], bf16, tag="ps_m")
                    nc.tensor.transpose(ps_vt[:], vm_bf[:], ident_bf[:])
                    vmt_bf = poold.tile([128, 128], bf16, tag="vmt")
                    nc.scalar.copy(vmt_bf[:], ps_vt[:])
                    ps_vn = pss.tile([128, 128], f32, tag="ps_m")
                    nc.tensor.matmul(ps_vn[:], vmt_bf[:], t5[:], start=True, stop=True)
                    vm_bf = poold.tile([128, 128], bf16, tag="vm")
                    nc.vector.tensor_scalar(vm_bf[:], ps_vn[:], 0.25, scalar2=None,
                                            op0=mybir.AluOpType.mult)
                ps_vt2 = pss.tile([128, 128], bf16, tag="ps_m")
                nc.tensor.transpose(ps_vt2[:], vm_bf[:], ident_bf[:])
                vmt2 = poold.tile([128, 128], bf16, tag="vmt2")
                nc.scalar.copy(vmt2[:], ps_vt2[:])
                ps_A = pss.tile([128, 64], f32, tag="ps_a")
                nc.tensor.matmul(ps_A[:], vmt2[:], s_bf[:], start=True, stop=True)
                b_bf = pool.tile([128, 65], bf16, tag="b_bf")
                nc.vector.memset(b_bf[:, 64:65], 1.0)
                nc.vector.tensor_copy(b_bf[:, 0:64], ps_A[:])

                for j in range(NCH):
                    ps_c = psum.tile([128, NCHUNK], f32, tag="ps_big")
                    nc.tensor.matmul(ps_c[:], nct_r[:], qt_r[:, j*NCHUNK:(j+1)*NCHUNK],
                                     start=True, stop=True)
                    ec = poold.tile([128, NCHUNK], bf16, tag="ec")
                    nc.scalar.activation(ec[:], ps_c[:], AF.Exp)
                    ps_X = psum.tile([128, 4, 65], f32, tag="ps_x")
                    for t in range(4):
                        nc.tensor.matmul(ps_X[:, t, :], ec[:, t*128:(t+1)*128], b_bf[:],
                                         start=True, stop=True)
                    xrec = poold.tile([128, 4], f32, tag="xrec")
                    nc.vector.reciprocal(xrec[:], ps_X[:, :, 64])
                    xout = poold.tile([128, 4, 64], f32, tag="xout")
                    nc.vector.tensor_tensor(out=xout[:], in0=ps_X[:, :, 0:64],
                        in1=xrec.rearrange("p (t o) -> p t o", o=1).to_broadcast([128, 4, 64]),
                        op=mybir.AluOpType.mult)
                    nc.sync.dma_start(
                        XO[p, j*NCHUNK:(j+1)*NCHUNK, :].rearrange("(t pp) d -> pp t d", pp=128),
                        xout[:])
    nc.finalize()
    _cache["nc"] = nc
    return nc


def kernel(Q, K, V, mask):
    from concourse.bass_utils import run_bass_kernel_spmd

    Q = np.asarray(Q, dtype=np.float32)
    K = np.asarray(K, dtype=np.float32)
    V = np.asarray(V, dtype=np.float32)
    Qf = Q.reshape(B * H, N, D)
    Kf = K.reshape(B * H, N, D)
    Vf = V.reshape(B * H, N, D)

    nct = np.empty((B * H, D, M), np.float32)
    nrt = np.empty((B * H, D, M), np.float32)
    gmax = 0.0
    for i in range(B * H):
        for (T, out) in ((Kf, nct), (Qf, nrt)):
            s = T[i, :, 0].copy()
            s[0] = np.inf
            idx = np.argpartition(-s, M)[:M]
            out[i] = T[i, np.sort(idx), :].T
        nr = nrt[i].T.astype(np.float64)
        nc_ = nct[i].T.astype(np.float64)
        m = nr @ nc_.T
        e = np.exp(m - m.max(axis=1, keepdims=True))
        k2 = e / e.sum(axis=1, keepdims=True)
        gmax = max(gmax, float(k2.sum(axis=0).max()))

    QTf = np.ascontiguousarray(Qf.transpose(0, 2, 1))
    KTf = np.ascontiguousarray(Kf.transpose(0, 2, 1))
    Vbf = Vf.astype(ml_dtypes.bfloat16)
    gs = np.array([[1.0 / gmax]], np.float32)

    nc = _build()
    in_maps = []
    for c in range(NCORES):
        sl = slice(c * PAIRS, (c + 1) * PAIRS)
        in_maps.append({"QT": QTf[sl], "KT": KTf[sl], "Vb": Vbf[sl],
                        "NCT": nct[sl], "NRT": nrt[sl], "GS": gs})
    res = run_bass_kernel_spmd(nc, in_maps, list(range(NCORES)))
    _cache["last_result"] = res
    X = np.concatenate([res.results[c]["XO"] for c in range(NCORES)], axis=0)
    return X.reshape(B, H, N, D)



# revision 7
# speedup vs baseline: 1.9583x; 1.9583x over previous
import numpy as np
import ml_dtypes

B, H, N, D = 4, 12, 8192, 64
M = 128
NCORES = 8
PAIRS = (B * H) // NCORES
NT = N // 128  # 64 column-blocks of 128

_cache = {}


def _build():
    if "nc" in _cache:
        return _cache["nc"]
    import concourse.bacc as bacc
    import concourse.mybir as mybir
    import concourse.tile as tile

    f32, f32r, bf16 = mybir.dt.float32, mybir.dt.float32r, mybir.dt.bfloat16
    AF = mybir.ActivationFunctionType
    MULT = mybir.AluOpType.mult
    SUB = mybir.AluOpType.subtract

    nc = bacc.Bacc("TRN2", target_bir_lowering=False, debug=False)
    QT = nc.declare_dram_parameter("QT", [PAIRS, 64, N], f32, isOutput=False)
    KTB = nc.declare_dram_parameter("KTB", [PAIRS, 64, N], bf16, isOutput=False)
    NRB = nc.declare_dram_parameter("NRB", [PAIRS, 64, M], bf16, isOutput=False)
    LM = nc.declare_dram_parameter("LM", [PAIRS, 2, 64, M], f32, isOutput=False)
    VB = nc.declare_dram_parameter("VB", [PAIRS, 128, NT, 65], bf16, isOutput=False)
    GS = nc.declare_dram_parameter("GS", [1, 1], f32, isOutput=False)
    XO = nc.declare_dram_parameter("XO", [PAIRS, N, 64], bf16, isOutput=True)

    RGROUPS = [4] * 16  # 64 r-blocks in groups (matches rt tile depth 4)

    with tile.TileContext(nc) as tc:
        with (tc.tile_pool(name="pc", bufs=1) as pc,
              tc.tile_pool(name="pio", bufs=2) as pio,
              tc.tile_pool(name="pw", bufs=3) as pw,
              tc.tile_pool(name="pn", bufs=2) as pn,
              tc.tile_pool(name="po", bufs=2) as po,
              tc.tile_pool(name="ps1", bufs=1, space="PSUM") as ps1,
              tc.tile_pool(name="psr", bufs=3, space="PSUM") as psr,
              tc.tile_pool(name="ps2", bufs=2, space="PSUM") as ps2):

            # ---- constants ----
            ident = pc.tile([128, 128], bf16, tag="ident")
            nc.gpsimd.memset(ident[:], 0.0)
            nc.gpsimd.affine_select(out=ident[:], in_=ident[:],
                compare_op=mybir.AluOpType.not_equal, fill=1.0, base=0,
                pattern=[[-1, 128]], channel_multiplier=1)
            diags = {}
            for val in (7, 15, 13):
                t = pc.tile([128, PAIRS, 128], bf16, tag=f"diag{val}")
                nc.gpsimd.memset(t[:], 0.0)
                for p in range(PAIRS):
                    nc.gpsimd.affine_select(out=t[:, p, :], in_=t[:, p, :],
                        compare_op=mybir.AluOpType.not_equal, fill=float(val), base=0,
                        pattern=[[-1, 128]], channel_multiplier=1)
                diags[val] = t
            ones_row = pc.tile([1, 128], f32, tag="ones_row")
            nc.vector.memset(ones_row[:], 1.0)
            gs_sb = pc.tile([1, 1], f32, tag="gs_sb")
            nc.sync.dma_start(gs_sb[:], GS[:])
            nsp0 = ps2.tile([128, 4, 128], f32, tag="ns")
            nc.tensor.matmul(nsp0[:, 0, 0:1], ones_row[:], gs_sb[:], start=True, stop=True)
            gsb = pc.tile([128, 1], f32, tag="gsb")
            nc.vector.tensor_copy(gsb[:], nsp0[:, 0, 0:1])

            # ---- batched NS state ----
            kmt6 = pc.tile([128, PAIRS, 128], bf16, tag="kmt6")
            vm6 = pc.tile([128, PAIRS, 128], bf16, tag="vm6")
            vmt6 = pc.tile([128, PAIRS, 128], bf16, tag="vmt6")
            ct6 = pc.tile([128, PAIRS, 128], bf16, tag="ct6")
            s_sb6 = pc.tile([128, PAIRS, 65], bf16, tag="s_sb6")
            rrec6 = pc.tile([128, PAIRS], f32, tag="rrec6")
            nr6 = pc.tile([64, PAIRS, M], f32r, tag="nr6")
            nc6 = pc.tile([64, PAIRS, M], f32r, tag="nc6")
            ps_s6 = ps1.tile([128, PAIRS, 85], f32, tag="s6")

            qts = {}

            def load_qt(p):
                t = pio.tile([64, N], f32r, tag="qt")
                nc.gpsimd.dma_start(t[:], QT[p])
                qts[p] = t

            def phase_a(p):
                ktb = pio.tile([64, N], bf16, tag="ktb")
                nrb = pio.tile([64, M], bf16, tag="nrb")
                vb = pio.tile([128, NT, 65], bf16, tag="vb")
                nc.sync.dma_start(ktb[:], KTB[p])
                nc.sync.dma_start(nrb[:], NRB[p])
                nc.gpsimd.dma_start(nr6[:, p, :], LM[p, 0])
                nc.gpsimd.dma_start(nc6[:, p, :], LM[p, 1])
                nc.sync.dma_start(vb[:], VB[p])

                # r-side: r^T blocks -> exp -> S/denominator accumulation
                pending = None
                t0 = 0
                for cnt in RGROUPS:
                    rt = psr.tile([128, 4, 128], f32, tag="rt")
                    for t in range(cnt):
                        nc.tensor.matmul(rt[:, t, :],
                                         ktb[:, (t0 + t) * 128:(t0 + t + 1) * 128],
                                         nrb[:], start=True, stop=True)
                    ert = pw.tile([128, 4, 128], bf16, tag="ert")
                    nc.scalar.activation(ert[:, 0:cnt, :], rt[:, 0:cnt, :], AF.Exp)
                    if pending is not None:
                        pert, pt0, pcnt = pending
                        for t in range(pcnt):
                            nc.tensor.matmul(ps_s6[:, p, 0:65], pert[:, t, :],
                                             vb[:, pt0 + t, :],
                                             start=(pt0 + t == 0), stop=False)
                    pending = (ert, t0, cnt)
                    t0 += cnt
                pert, pt0, pcnt = pending
                for t in range(pcnt):
                    nc.tensor.matmul(ps_s6[:, p, 0:65], pert[:, t, :],
                                     vb[:, pt0 + t, :],
                                     start=False, stop=(pt0 + t == NT - 1))

                # m / k2 / NS init
                nsp = ps2.tile([128, 4, 128], f32, tag="ns")
                nc.tensor.matmul(nsp[:, 0, :], nr6[:, p, :], nc6[:, p, :],
                                 start=True, stop=True)
                e_m = pw.tile([128, 128], bf16, tag="em")
                msum = pw.tile([128, 1], f32, tag="msum")
                nc.scalar.activation(e_m[:], nsp[:, 0, :], AF.Exp, accum_out=msum[:])
                mrec = pw.tile([128, 1], f32, tag="mrec")
                nc.vector.reciprocal(mrec[:], msum[:])
                k2n = pw.tile([128, 128], bf16, tag="k2n")
                nc.vector.tensor_scalar_mul(k2n[:], e_m[:], mrec[:])
                nsp2 = ps2.tile([128, 4, 128], f32, tag="ns")
                nc.tensor.matmul(nsp2[:, 0, :], k2n[:], ident[:], start=True, stop=True)
                nc.vector.tensor_copy(kmt6[:, p, :], nsp2[:, 0, :])
                nc.vector.tensor_scalar_mul(vm6[:, p, :], nsp2[:, 0, :], gsb[:])
                nc.vector.tensor_scalar_mul(vmt6[:, p, :], k2n[:], gsb[:])

                # stash r denominators
                nc.vector.tensor_copy(s_sb6[:, p, :], ps_s6[:, p, 0:65])
                nc.vector.reciprocal(rrec6[:, p:p + 1], ps_s6[:, p, 64:65])

            def ns_group(g):
                prs = [3 * g, 3 * g + 1, 3 * g + 2]
                dsl = slice(3 * g, 3 * g + 3)
                for it in range(6):
                    e_ps = ps2.tile([128, 4, 128], f32, tag="ns")
                    for k, p in enumerate(prs):
                        nc.tensor.matmul(e_ps[:, k, :], kmt6[:, p, :], vm6[:, p, :],
                                         start=True, stop=True)
                    et_ps = ps2.tile([128, 4, 128], f32, tag="ns")
                    for k, p in enumerate(prs):
                        nc.tensor.matmul(et_ps[:, k, :], vm6[:, p, :], kmt6[:, p, :],
                                         start=True, stop=True)
                    g1 = pn.tile([128, 3, 128], bf16, tag="g1")
                    nc.vector.tensor_tensor(out=g1[:], in0=diags[7][:, dsl, :],
                                            in1=e_ps[:, 0:3, :], op=SUB)
                    et = pn.tile([128, 3, 128], bf16, tag="et")
                    nc.vector.tensor_copy(et[:], et_ps[:, 0:3, :])
                    p2_ps = ps2.tile([128, 4, 128], f32, tag="ns")
                    for k in range(3):
                        nc.tensor.matmul(p2_ps[:, k, :], et[:, k, :], g1[:, k, :],
                                         start=True, stop=True)
                    g2 = pn.tile([128, 3, 128], bf16, tag="g2")
                    nc.vector.tensor_tensor(out=g2[:], in0=diags[15][:, dsl, :],
                                            in1=p2_ps[:, 0:3, :], op=SUB)
                    p3_ps = ps2.tile([128, 4, 128], f32, tag="ns")
                    for k in range(3):
                        nc.tensor.matmul(p3_ps[:, k, :], et[:, k, :], g2[:, k, :],
                                         start=True, stop=True)
                    g3 = pn.tile([128, 3, 128], bf16, tag="g3")
                    nc.vector.tensor_tensor(out=g3[:], in0=diags[13][:, dsl, :],
                                            in1=p3_ps[:, 0:3, :], op=SUB)
                    if it < 5:
                        v_ps = ps2.tile([128, 4, 128], f32, tag="ns")
                        for k, p in enumerate(prs):
                            nc.tensor.matmul(v_ps[:, k, :], vmt6[:, p, :], g3[:, k, :],
                                             start=True, stop=True)
                        vt_ps = ps2.tile([128, 4, 128], f32, tag="ns")
                        for k, p in enumerate(prs):
                            nc.tensor.matmul(vt_ps[:, k, :], g3[:, k, :], vmt6[:, p, :],
                                             start=True, stop=True)
                        nc.vector.tensor_scalar(vm6[:, dsl, :], v_ps[:, 0:3, :],
                                                0.25, scalar2=None, op0=MULT)
                        nc.vector.tensor_scalar(vmt6[:, dsl, :], vt_ps[:, 0:3, :],
                                                0.25, scalar2=None, op0=MULT)
                    else:
                        vt_ps = ps2.tile([128, 4, 128], f32, tag="ns")
                        for k, p in enumerate(prs):
                            nc.tensor.matmul(vt_ps[:, k, :], g3[:, k, :], vmt6[:, p, :],
                                             start=True, stop=True)
                        for k, p in enumerate(prs):
                            nc.vector.tensor_scalar(ct6[:, p, :], vt_ps[:, k, :],
                                                    rrec6[:, p:p + 1], scalar2=0.25,
                                                    op0=MULT, op1=MULT)

            def phase_b(p):
                if p + 2 < PAIRS:
                    load_qt(p + 2)
                qt_r = qts[p]
                a_ps = ps2.tile([128, 4, 128], f32, tag="ns")
                nc.tensor.matmul(a_ps[:, 0, 0:65], ct6[:, p, :], s_sb6[:, p, :],
                                 start=True, stop=True)
                b_sb = pw.tile([128, 65], bf16, tag="bsb")
                nc.vector.memset(b_sb[:, 64:65], 1.0)
                nc.vector.tensor_copy(b_sb[:, 0:64], a_ps[:, 0, 0:64])

                for j in range(8):  # 8 groups of 1024 N-columns
                    xo = po.tile([128, 2, 4, 64], bf16, tag="xo")
                    for h in range(2):
                        n0 = j * 1024 + h * 512
                        cp = ps1.tile([128, 4, 128], f32, tag=f"c{h}")
                        nc.tensor.matmul(cp[:].rearrange("a b c -> a (b c)"),
                                         nc6[:, p, :], qt_r[:, n0:n0 + 512],
                                         start=True, stop=True)
                        ec = pw.tile([128, 128, 4], bf16, tag="ec")
                        nc.scalar.activation(
                            ec[:].rearrange("a b c -> a (b c)"),
                            cp[:].rearrange("a b c -> a (b c)"), AF.Exp)
                        xp = ps2.tile([128, 4, 128], f32, tag="ns")
                        for k in range(4):
                            nc.tensor.matmul(xp[:, k, 0:65], ec[:, :, k], b_sb[:],
                                             start=True, stop=True)
                        xr = pw.tile([128, 4], f32, tag="xr")
                        nc.vector.reciprocal(xr[:], xp[:, 0:4, 64])
                        nc.vector.tensor_tensor(
                            out=xo[:, h, :, :], in0=xp[:, 0:4, 0:64],
                            in1=xr.rearrange("a (b c) -> a b c", c=1)
                                  .to_broadcast([128, 4, 64]),
                            op=MULT)
                    nc.sync.dma_start(
                        XO[p, j * 1024:(j + 1) * 1024, :]
                        .rearrange("(h pp r) d -> pp h (r d)", h=2, pp=128),
                        xo[:].rearrange("a h r d -> a h (r d)"))

            for p in range(PAIRS):
                if p < 2:
                    load_qt(p)
                phase_a(p)
                if p == 2:
                    ns_group(0)
                if p == 5:
                    ns_group(1)
            for p in range(PAIRS):
                phase_b(p)
    nc.finalize()
    _cache["nc"] = nc
    return nc


def kernel(Q, K, V, mask):
    from concourse.bass_utils import run_bass_kernel_spmd

    Q = np.asarray(Q, dtype=np.float32)
    K = np.asarray(K, dtype=np.float32)
    V = np.asarray(V, dtype=np.float32)
    BH = B * H
    Qf = Q.reshape(BH, N, D)
    Kf = K.reshape(BH, N, D)
    Vf = V.reshape(BH, N, D)

    nct = np.empty((BH, D, M), np.float32)
    nrt = np.empty((BH, D, M), np.float32)
    gmax = 0.0
    for i in range(BH):
        for (T, out) in ((Kf, nct), (Qf, nrt)):
            s = T[i, :, 0].copy()
            s[0] = np.inf
            idx = np.argpartition(-s, M)[:M]
            out[i] = T[i, np.sort(idx), :].T
        nr = nrt[i].T.astype(np.float64)
        nc_ = nct[i].T.astype(np.float64)
        m = nr @ nc_.T
        e = np.exp(m - m.max(axis=1, keepdims=True))
        k2 = e / e.sum(axis=1, keepdims=True)
        gmax = max(gmax, float(k2.sum(axis=0).max()))

    QTf = np.ascontiguousarray(Qf.transpose(0, 2, 1))
    KTBf = np.ascontiguousarray(Kf.transpose(0, 2, 1)).astype(ml_dtypes.bfloat16)
    NRBf = nrt.astype(ml_dtypes.bfloat16)
    LMf = np.stack([nrt, nct], axis=1)  # [BH, 2, 64, M] f32
    Vb = np.empty((BH, 128, NT, 65), np.float32)
    Vb[:, :, :, 64] = 1.0
    Vb[:, :, :, 0:64] = Vf.reshape(BH, NT, 128, D).transpose(0, 2, 1, 3)
    VBf = Vb.astype(ml_dtypes.bfloat16)
    gs = np.array([[1.0 / gmax]], np.float32)

    nc = _build()
    in_maps = []
    for c in range(NCORES):
        sl = slice(c * PAIRS, (c + 1) * PAIRS)
        in_maps.append({"QT": QTf[sl], "KTB": KTBf[sl], "NRB": NRBf[sl],
                        "LM": LMf[sl], "VB": VBf[sl], "GS": gs})
    res = run_bass_kernel_spmd(nc, in_maps, list(range(NCORES)))
    _cache["last_result"] = res
    X = np.concatenate([res.results[c]["XO"] for c in range(NCORES)], axis=0)
    return X.astype(np.float32).reshape(B, H, N, D)


# revision 27
# speedup vs baseline: 2.6999x; 1.3787x over previous
import numpy as np
import ml_dtypes

B, H, N, D = 4, 12, 8192, 64
M = 128
NCORES = 8
PAIRS = (B * H) // NCORES
NT = N // 128  # 64 column-blocks of 128

_cache = {}


def _build():
    if "nc" in _cache:
        return _cache["nc"]
    import concourse.bacc as bacc
    import concourse.mybir as mybir
    import concourse.tile as tile

    f32, f32r, bf16 = mybir.dt.float32, mybir.dt.float32r, mybir.dt.bfloat16
    AF = mybir.ActivationFunctionType
    MULT = mybir.AluOpType.mult
    SUB = mybir.AluOpType.subtract

    nc = bacc.Bacc("TRN2", target_bir_lowering=False, debug=False)
    QT = nc.declare_dram_parameter("QT", [PAIRS, 64, N], f32, isOutput=False)
    KTB = nc.declare_dram_parameter("KTB", [PAIRS, 64, N], bf16, isOutput=False)
    NRB = nc.declare_dram_parameter("NRB", [PAIRS, 64, M], bf16, isOutput=False)
    LM = nc.declare_dram_parameter("LM", [PAIRS, 2, 64, M], f32, isOutput=False)
    VB = nc.declare_dram_parameter("VB", [PAIRS, 128, NT, 65], bf16, isOutput=False)
    GS = nc.declare_dram_parameter("GS", [1, 1], f32, isOutput=False)
    XO = nc.declare_dram_parameter("XO", [PAIRS, N, 64], bf16, isOutput=True)

    RGROUPS = [4] * 16  # 64 r-blocks in groups (matches rt tile depth 4)

    with tile.TileContext(nc) as tc:
        with (tc.tile_pool(name="pc", bufs=1) as pc,
              tc.tile_pool(name="pio", bufs=2) as pio,
              tc.tile_pool(name="pw", bufs=6) as pw,
              tc.tile_pool(name="pn", bufs=3) as pn,
              tc.tile_pool(name="po", bufs=6) as po,
              tc.tile_pool(name="ps1", bufs=1, space="PSUM") as ps1,
              tc.tile_pool(name="psr", bufs=3, space="PSUM") as psr,
              tc.tile_pool(name="ps2", bufs=2, space="PSUM") as ps2):

            # ---- preload pair 0 (DMA starts before const setup) ----
            pre = {}
            pre["ktb"] = pio.tile([64, N], bf16, tag="ktb", name="ktb0")
            pre["nrb"] = pio.tile([64, M], bf16, tag="nrb", name="nrb0")
            pre["vb"] = pio.tile([128, NT, 65], bf16, tag="vb", name="vb0")
            nc.sync.dma_start(pre["ktb"][:, 0:N // 2], KTB[0, :, 0:N // 2])
            nc.sync.dma_start(pre["ktb"][:, N // 2:], KTB[0, :, N // 2:])
            nc.sync.dma_start(pre["nrb"][:], NRB[0])
            nc.sync.dma_start(pre["vb"][:], VB[0])

            # ---- constants ----
            ident = pc.tile([128, 128], bf16, tag="ident")
            nc.gpsimd.memset(ident[:], 0.0)
            nc.gpsimd.affine_select(out=ident[:], in_=ident[:],
                compare_op=mybir.AluOpType.not_equal, fill=1.0, base=0,
                pattern=[[-1, 128]], channel_multiplier=1)
            diags = {}
            for val in (7, 15, 13):
                t = pc.tile([128, PAIRS, 128], bf16, tag=f"diag{val}")
                nc.gpsimd.memset(t[:], 0.0)
                for p in range(PAIRS):
                    nc.gpsimd.affine_select(out=t[:, p, :], in_=t[:, p, :],
                        compare_op=mybir.AluOpType.not_equal, fill=float(val), base=0,
                        pattern=[[-1, 128]], channel_multiplier=1)
                diags[val] = t
            ones_row = pc.tile([1, 128], f32, tag="ones_row")
            nc.vector.memset(ones_row[:], 1.0)
            gs_sb = pc.tile([1, 1], f32, tag="gs_sb")
            nc.sync.dma_start(gs_sb[:], GS[:])
            nsp0 = ps2.tile([128, 4, 128], f32, tag="ns")
            nc.tensor.matmul(nsp0[:, 0, 0:1], ones_row[:], gs_sb[:], start=True, stop=True)
            gsb = pc.tile([128, 1], f32, tag="gsb")
            nc.vector.tensor_copy(gsb[:], nsp0[:, 0, 0:1])

            # ---- batched NS state ----
            kmt6 = pc.tile([128, PAIRS, 128], bf16, tag="kmt6")
            vm6 = pc.tile([128, PAIRS, 128], bf16, tag="vm6")
            vmt6 = pc.tile([128, PAIRS, 128], bf16, tag="vmt6")
            ct6 = pc.tile([128, PAIRS, 128], bf16, tag="ct6")
            s_sb6 = pc.tile([128, PAIRS, 65], bf16, tag="s_sb6")
            rrec6 = pc.tile([128, PAIRS], f32, tag="rrec6")
            nr6 = pc.tile([64, PAIRS, M], f32r, tag="nr6")
            nc6 = pc.tile([64, PAIRS, M], f32r, tag="nc6")
            ps_s6 = ps1.tile([128, PAIRS, 85], f32, tag="s6")

            qts = {}

            def load_qt(p):
                t = pio.tile([64, N], f32r, tag="qt")
                for q in range(4):
                    nc.gpsimd.dma_start(t[:, q * (N // 4):(q + 1) * (N // 4)],
                                        QT[p, :, q * (N // 4):(q + 1) * (N // 4)])
                qts[p] = t

            def phase_a(p):
                ktb = pio.tile([64, N], bf16, tag="ktb")
                nrb = pio.tile([64, M], bf16, tag="nrb")
                vb = pio.tile([128, NT, 65], bf16, tag="vb")
                nc.sync.dma_start(ktb[:], KTB[p])
                nc.sync.dma_start(nrb[:], NRB[p])
                nc.gpsimd.dma_start(nr6[:, p, :], LM[p, 0])
                nc.gpsimd.dma_start(nc6[:, p, :], LM[p, 1])
                nc.sync.dma_start(vb[:], VB[p])

                # r-side: r^T blocks -> exp -> S/denominator accumulation
                pending = None
                t0 = 0
                for cnt in RGROUPS:
                    rt = psr.tile([128, 4, 128], f32, tag="rt")
                    for t in range(cnt):
                        nc.tensor.matmul(rt[:, t, :],
                                         ktb[:, (t0 + t) * 128:(t0 + t + 1) * 128],
                                         nrb[:], start=True, stop=True)
                    ert = pw.tile([128, 4, 128], bf16, tag="ert")
                    nc.scalar.activation(ert[:, 0:cnt, :], rt[:, 0:cnt, :], AF.Exp)
                    if pending is not None:
                        pert, pt0, pcnt = pending
                        for t in range(pcnt):
                            nc.tensor.matmul(ps_s6[:, p, 0:65], pert[:, t, :],
                                             vb[:, pt0 + t, :],
                                             start=(pt0 + t == 0), stop=False)
                    pending = (ert, t0, cnt)
                    t0 += cnt
                pert, pt0, pcnt = pending
                for t in range(pcnt):
                    nc.tensor.matmul(ps_s6[:, p, 0:65], pert[:, t, :],
                                     vb[:, pt0 + t, :],
                                     start=False, stop=(pt0 + t == NT - 1))

                # m / k2 / NS init
                nsp = ps2.tile([128, 4, 128], f32, tag="ns")
                nc.tensor.matmul(nsp[:, 0, :], nr6[:, p, :], nc6[:, p, :],
                                 start=True, stop=True)
                e_m = pw.tile([128, 128], bf16, tag="em")
                msum = pw.tile([128, 1], f32, tag="msum")
                nc.scalar.activation(e_m[:], nsp[:, 0, :], AF.Exp, accum_out=msum[:])
                mrec = pw.tile([128, 1], f32, tag="mrec")
                nc.vector.reciprocal(mrec[:], msum[:])
                k2n = pw.tile([128, 128], bf16, tag="k2n")
                nc.vector.tensor_scalar_mul(k2n[:], e_m[:], mrec[:])
                nsp2 = ps2.tile([128, 4, 128], f32, tag="ns")
                nc.tensor.matmul(nsp2[:, 0, :], k2n[:], ident[:], start=True, stop=True)
                nc.vector.tensor_copy(kmt6[:, p, :], nsp2[:, 0, :])
                nc.vector.tensor_scalar_mul(vm6[:, p, :], nsp2[:, 0, :], gsb[:])
                nc.vector.tensor_scalar_mul(vmt6[:, p, :], k2n[:], gsb[:])

                # stash r denominators
                nc.vector.tensor_copy(s_sb6[:, p, :], ps_s6[:, p, 0:65])
                nc.vector.reciprocal(rrec6[:, p:p + 1], ps_s6[:, p, 64:65])

            def ns_group(g):
                prs = [2 * g, 2 * g + 1]
                dsl = slice(2 * g, 2 * g + 2)
                for it in range(6):
                    e_ps = ps2.tile([128, 4, 128], f32, tag="ns")
                    for k, p in enumerate(prs):
                        nc.tensor.matmul(e_ps[:, k, :], kmt6[:, p, :], vm6[:, p, :],
                                         start=True, stop=True)
                    et_ps = ps2.tile([128, 4, 128], f32, tag="ns")
                    for k, p in enumerate(prs):
                        nc.tensor.matmul(et_ps[:, k, :], vm6[:, p, :], kmt6[:, p, :],
                                         start=True, stop=True)
                    g1 = pn.tile([128, ng, 128], bf16, tag="g1")
                    nc.vector.tensor_tensor(out=g1[:], in0=diags[7][:, dsl, :],
                                            in1=e_ps[:, 0:ng, :], op=SUB)
                    et = pn.tile([128, ng, 128], bf16, tag="et")
                    if act_copies:
                        nc.scalar.activation(et[:], et_ps[:, 0:ng, :], AF.Copy)
                    else:
                        nc.vector.tensor_copy(et[:], et_ps[:, 0:ng, :])
                    p2_ps = ps2.tile([128, 4, 128], f32, tag="ns")
                    for k in range(ng):
                        nc.tensor.matmul(p2_ps[:, k, :], et[:, k, :], g1[:, k, :],
                                         start=True, stop=True)
                    g2 = pn.tile([128, ng, 128], bf16, tag="g2")
                    nc.vector.tensor_tensor(out=g2[:], in0=diags[15][:, dsl, :],
                                            in1=p2_ps[:, 0:ng, :], op=SUB)
                    p3_ps = ps2.tile([128, 4, 128], f32, tag="ns")
                    for k in range(ng):
                        nc.tensor.matmul(p3_ps[:, k, :], et[:, k, :], g2[:, k, :],
                                         start=True, stop=True)
                    g3 = pn.tile([128, ng, 128], bf16, tag="g3")
                    nc.vector.tensor_tensor(out=g3[:], in0=diags[13][:, dsl, :],
                                            in1=p3_ps[:, 0:ng, :], op=SUB)
                    if it < 5:
                        v_ps = ps2.tile([128, 4, 128], f32, tag="ns")
                        for k, p in enumerate(prs):
                            nc.tensor.matmul(v_ps[:, k, :], vmt6[:, p, :], g3[:, k, :],
                                             start=True, stop=True)
                        vt_ps = ps2.tile([128, 4, 128], f32, tag="ns")
                        for k, p in enumerate(prs):
                            nc.tensor.matmul(vt_ps[:, k, :], g3[:, k, :], vmt6[:, p, :],
                                             start=True, stop=True)
                        if act_copies:
                            nc.scalar.activation(vm6[:, dsl, :], v_ps[:, 0:ng, :],
                                                 AF.Copy, scale=0.25)
                            nc.scalar.activation(vmt6[:, dsl, :], vt_ps[:, 0:ng, :],
                                                 AF.Copy, scale=0.25)
                        else:
                            nc.vector.tensor_scalar(vm6[:, dsl, :], v_ps[:, 0:ng, :],
                                                    0.25, scalar2=None, op0=MULT)
                            nc.vector.tensor_scalar(vmt6[:, dsl, :], vt_ps[:, 0:ng, :],
                                                    0.25, scalar2=None, op0=MULT)
                    else:
                        vt_ps = ps2.tile([128, 4, 128], f32, tag="ns")
                        for k, p in enumerate(prs):
                            nc.tensor.matmul(vt_ps[:, k, :], g3[:, k, :], vmt6[:, p, :],
                                             start=True, stop=True)
                        for k, p in enumerate(prs):
                            nc.vector.tensor_scalar(ct6[:, p, :], vt_ps[:, k, :],
                                                    rrec6[:, p:p + 1], scalar2=0.25,
                                                    op0=MULT, op1=MULT)

            def phase_b_all():
                """Flat generator over all pairs; lag-2 queue crosses pairs."""
                xo_tiles = {}

                def do_out(p, j, h, ec, b_sb):
                    if h == 0:
                        xo_new = po.tile([128, 2, 4, 64], bf16, tag="xo")
                        xo_tiles[(p, j)] = xo_new
                    xo = xo_tiles[(p, j)]
                    xp = ps2.tile([128, 4, 128], f32, tag="ns")
                    for k in range(4):
                        nc.tensor.matmul(xp[:, k, 0:65], ec[:, :, k], b_sb[:],
                                         start=True, stop=True)
                    xr = pw.tile([128, 4], f32, tag="xr")
                    nc.vector.reciprocal(xr[:], xp[:, 0:4, 64])
                    nc.vector.tensor_tensor(
                        out=xo[:, h, :, :], in0=xp[:, 0:4, 0:64],
                        in1=xr.rearrange("a (b c) -> a b c", c=1)
                              .to_broadcast([128, 4, 64]),
                        op=MULT)
                    if h == 1:
                        nc.sync.dma_start(
                            XO[p, j * 1024:(j + 1) * 1024, :]
                            .rearrange("(h pp r) d -> pp h (r d)", h=2, pp=128),
                            xo[:].rearrange("a h r d -> a h (r d)"))
                        del xo_tiles[(p, j)]

                pend = []
                for p in range(PAIRS):
                    if p + 2 < PAIRS:
                        load_qt(p + 2)
                    qt_r = qts[p]
                    a_ps = ps2.tile([128, 4, 128], f32, tag="ns")
                    nc.tensor.matmul(a_ps[:, 0, 0:65], ct6[:, p, :], s_sb6[:, p, :],
                                     start=True, stop=True)
                    b_sb = pw.tile([128, 65], bf16, tag="bsb")
                    nc.vector.memset(b_sb[:, 64:65], 1.0)
                    nc.vector.tensor_copy(b_sb[:, 0:64], a_ps[:, 0, 0:64])
                    for j in range(8):
                        for h in range(2):
                            yield
                            n0 = j * 1024 + h * 512
                            cp = ps1.tile([128, 4, 128], f32, tag=f"c{h}")
                            nc.tensor.matmul(cp[:].rearrange("a b c -> a (b c)"),
                                             nc6[:, p, :], qt_r[:, n0:n0 + 512],
                                             start=True, stop=True)
                            ec = pw.tile([128, 128, 4], bf16, tag="ec")
                            nc.scalar.activation(
                                ec[:].rearrange("a b c -> a (b c)"),
                                cp[:].rearrange("a b c -> a (b c)"), AF.Exp)
                            pend.append((p, j, h, ec, b_sb))
                            if len(pend) > 2:
                                do_out(*pend.pop(0))
                for it in pend:
                    do_out(*it)

            def drive(primary, aux):
                for _ in primary:
                    if aux is not None:
                        next(aux, None)

            import os
            NSCFG = os.environ.get("NSCFG", "1")
            nsg = None
            for p in range(PAIRS):
                if p < 2:
                    load_qt(p)
                if p == 3:
                    nsg = ns_group([0, 1, 2])
                if p == 5 and NSCFG == "2":
                    for _ in nsg:
                        pass
                    nsg = ns_group([3, 4])
                drive(phase_a(p), nsg if p >= 3 else None)
            if nsg is not None:
                for _ in nsg:
                    pass
            if NSCFG == "2":
                drive(phase_b_all(), ns_group([5]))
            else:
                drive(phase_b_all(), ns_group([3, 4, 5]))
    nc.finalize()
    _cache["nc"] = nc
    return nc


def kernel(Q, K, V, mask):
    from concourse.bass_utils import run_bass_kernel_spmd

    Q = np.asarray(Q, dtype=np.float32)
    K = np.asarray(K, dtype=np.float32)
    V = np.asarray(V, dtype=np.float32)
    BH = B * H
    Qf = Q.reshape(BH, N, D)
    Kf = K.reshape(BH, N, D)
    Vf = V.reshape(BH, N, D)

    nct = np.empty((BH, D, M), np.float32)
    nrt = np.empty((BH, D, M), np.float32)
    gmax = 0.0
    for i in range(BH):
        for (T, out) in ((Kf, nct), (Qf, nrt)):
            s = T[i, :, 0].copy()
            s[0] = np.inf
            idx = np.argpartition(-s, M)[:M]
            out[i] = T[i, np.sort(idx), :].T
        nr = nrt[i].T.astype(np.float64)
        nc_ = nct[i].T.astype(np.float64)
        m = nr @ nc_.T
        e = np.exp(m - m.max(axis=1, keepdims=True))
        k2 = e / e.sum(axis=1, keepdims=True)
        gmax = max(gmax, float(k2.sum(axis=0).max()))

    QTf = np.ascontiguousarray(Qf.transpose(0, 2, 1))
    KTBf = np.ascontiguousarray(Kf.transpose(0, 2, 1)).astype(ml_dtypes.bfloat16)
    NRBf = nrt.astype(ml_dtypes.bfloat16)
    LMf = np.stack([nrt, nct], axis=1)  # [BH, 2, 64, M] f32
    Vb = np.empty((BH, 128, NT, 65), np.float32)
    Vb[:, :, :, 64] = 1.0
    Vb[:, :, :, 0:64] = Vf.reshape(BH, NT, 128, D).transpose(0, 2, 1, 3)
    VBf = Vb.astype(ml_dtypes.bfloat16)
    gs = np.array([[1.0 / gmax]], np.float32)

    nc = _build()
    in_maps = []
    for c in range(NCORES):
        sl = slice(c * PAIRS, (c + 1) * PAIRS)
        in_maps.append({"QT": QTf[sl], "KTB": KTBf[sl], "NRB": NRBf[sl],
                        "LM": LMf[sl], "VB": VBf[sl], "GS": gs})
    res = run_bass_kernel_spmd(nc, in_maps, list(range(NCORES)))
    _cache["last_result"] = res
    X = np.concatenate([res.results[c]["XO"] for c in range(NCORES)], axis=0)
    return X.astype(np.float32).reshape(B, H, N, D)


# revision 41
# speedup vs baseline: 2.8552x; 1.0575x over previous
import numpy as np
import ml_dtypes

B, H, N, D = 4, 12, 8192, 64
M = 128
NCORES = 8
PAIRS = (B * H) // NCORES
NT = N // 128  # 64 column-blocks of 128

_cache = {}


def _build():
    if "nc" in _cache:
        return _cache["nc"]
    import concourse.bacc as bacc
    import concourse.mybir as mybir
    import concourse.tile as tile

    f32, f32r, bf16 = mybir.dt.float32, mybir.dt.float32r, mybir.dt.bfloat16
    AF = mybir.ActivationFunctionType
    MULT = mybir.AluOpType.mult
    SUB = mybir.AluOpType.subtract

    nc = bacc.Bacc("TRN2", target_bir_lowering=False, debug=False)
    QT = nc.declare_dram_parameter("QT", [PAIRS, 64, N], f32, isOutput=False)
    KTB = nc.declare_dram_parameter("KTB", [PAIRS, 64, N], bf16, isOutput=False)
    NRB = nc.declare_dram_parameter("NRB", [PAIRS, 64, M], bf16, isOutput=False)
    LM = nc.declare_dram_parameter("LM", [PAIRS, 2, 64, M], f32, isOutput=False)
    VB = nc.declare_dram_parameter("VB", [PAIRS, 128, NT, 65], bf16, isOutput=False)
    GS = nc.declare_dram_parameter("GS", [1, 1], f32, isOutput=False)
    XO = nc.declare_dram_parameter("XO", [PAIRS, N, 64], bf16, isOutput=True)

    RGROUPS = [4] * 16  # 64 r-blocks in groups (matches rt tile depth 4)

    with tile.TileContext(nc) as tc:
        with (tc.tile_pool(name="pc", bufs=1) as pc,
              tc.tile_pool(name="pio", bufs=2) as pio,
              tc.tile_pool(name="pq", bufs=3) as pq,
              tc.tile_pool(name="pw", bufs=8) as pw,
              tc.tile_pool(name="pn", bufs=4) as pn,
              tc.tile_pool(name="po", bufs=8) as po,
              tc.tile_pool(name="ps1", bufs=1, space="PSUM") as ps1,
              tc.tile_pool(name="psr", bufs=3, space="PSUM") as psr,
              tc.tile_pool(name="ps2", bufs=3, space="PSUM") as ps2):

            # ---- preload pair 0 (DMA starts before const setup) ----
            pre = {}
            pre["ktb"] = pio.tile([64, N], bf16, tag="ktb", name="ktb0")
            pre["nrb"] = pio.tile([64, M], bf16, tag="nrb", name="nrb0")
            pre["vb"] = pio.tile([128, NT, 65], bf16, tag="vb", name="vb0")
            nc.sync.dma_start(pre["nrb"][:], NRB[0])
            for q in range(4):
                nc.sync.dma_start(pre["ktb"][:, q * (N // 4):(q + 1) * (N // 4)],
                                  KTB[0, :, q * (N // 4):(q + 1) * (N // 4)])
            nc.sync.dma_start(pre["vb"][:], VB[0])

            # ---- constants ----
            ident = pc.tile([128, 128], bf16, tag="ident")
            nc.gpsimd.memset(ident[:], 0.0)
            nc.gpsimd.affine_select(out=ident[:], in_=ident[:],
                compare_op=mybir.AluOpType.not_equal, fill=1.0, base=0,
                pattern=[[-1, 128]], channel_multiplier=1)
            diags = {}
            for val in (7, 15, 13):
                t = pc.tile([128, PAIRS, 128], bf16, tag=f"diag{val}")
                nc.gpsimd.memset(t[:], 0.0)
                for p in range(PAIRS):
                    nc.gpsimd.affine_select(out=t[:, p, :], in_=t[:, p, :],
                        compare_op=mybir.AluOpType.not_equal, fill=float(val), base=0,
                        pattern=[[-1, 128]], channel_multiplier=1)
                diags[val] = t
            ones_row = pc.tile([1, 128], f32, tag="ones_row")
            nc.vector.memset(ones_row[:], 1.0)
            gs_sb = pc.tile([1, 1], f32, tag="gs_sb")
            nc.sync.dma_start(gs_sb[:], GS[:])
            nsp0 = ps2.tile([128, 4, 128], f32, tag="ns")
            nc.tensor.matmul(nsp0[:, 0, 0:1], ones_row[:], gs_sb[:], start=True, stop=True)
            gsb = pc.tile([128, 1], f32, tag="gsb")
            nc.vector.tensor_copy(gsb[:], nsp0[:, 0, 0:1])

            # ---- batched NS state ----
            kmt6 = pc.tile([128, PAIRS, 128], bf16, tag="kmt6")
            vm6 = pc.tile([128, PAIRS, 128], bf16, tag="vm6")
            vmt6 = pc.tile([128, PAIRS, 128], bf16, tag="vmt6")
            ct6 = pc.tile([128, PAIRS, 128], bf16, tag="ct6")
            s_sb6 = pc.tile([128, PAIRS, 65], bf16, tag="s_sb6")
            rrec6 = pc.tile([128, PAIRS], f32, tag="rrec6")
            nr6 = pc.tile([64, PAIRS, M], f32r, tag="nr6")
            nc6 = pc.tile([64, PAIRS, M], f32r, tag="nc6")
            ps_share = ps1.tile([128, 512], f32, tag="share")

            qts = {}

            def load_qt(p, defer=False):
                t = pq.tile([64, N], f32r, tag="qt")
                qts[p] = t
                if not defer:
                    for q in range(4):
                        qt_chunk(p, q)

            def qt_chunk(p, q):
                t = qts[p]
                nc.gpsimd.dma_start(t[:, q * (N // 4):(q + 1) * (N // 4)],
                                    QT[p, :, q * (N // 4):(q + 1) * (N // 4)])

            def phase_a(p):
                ktb = pio.tile([64, N], bf16, tag="ktb")
                nrb = pio.tile([64, M], bf16, tag="nrb")
                vb = pio.tile([128, NT, 65], bf16, tag="vb")
                nc.sync.dma_start(ktb[:], KTB[p])
                nc.sync.dma_start(nrb[:], NRB[p])
                nc.gpsimd.dma_start(nr6[:, p, :], LM[p, 0])
                nc.gpsimd.dma_start(nc6[:, p, :], LM[p, 1])
                nc.sync.dma_start(vb[:], VB[p])

                # r-side: r^T blocks -> exp -> S/denominator accumulation
                pending = None
                t0 = 0
                for cnt in RGROUPS:
                    rt = psr.tile([128, 4, 128], f32, tag="rt")
                    for t in range(cnt):
                        nc.tensor.matmul(rt[:, t, :],
                                         ktb[:, (t0 + t) * 128:(t0 + t + 1) * 128],
                                         nrb[:], start=True, stop=True)
                    ert = pw.tile([128, 4, 128], bf16, tag="ert")
                    nc.scalar.activation(ert[:, 0:cnt, :], rt[:, 0:cnt, :], AF.Exp)
                    if pending is not None:
                        pert, pt0, pcnt = pending
                        for t in range(pcnt):
                            nc.tensor.matmul(ps_s6[:, p, 0:65], pert[:, t, :],
                                             vb[:, pt0 + t, :],
                                             start=(pt0 + t == 0), stop=False)
                    pending = (ert, t0, cnt)
                    t0 += cnt
                pert, pt0, pcnt = pending
                for t in range(pcnt):
                    nc.tensor.matmul(ps_s6[:, p, 0:65], pert[:, t, :],
                                     vb[:, pt0 + t, :],
                                     start=False, stop=(pt0 + t == NT - 1))

                # m / k2 / NS init
                nsp = ps2.tile([128, 4, 128], f32, tag="ns")
                nc.tensor.matmul(nsp[:, 0, :], nr6[:, p, :], nc6[:, p, :],
                                 start=True, stop=True)
                e_m = pw.tile([128, 128], bf16, tag="em")
                msum = pw.tile([128, 1], f32, tag="msum")
                nc.scalar.activation(e_m[:], nsp[:, 0, :], AF.Exp, accum_out=msum[:])
                mrec = pw.tile([128, 1], f32, tag="mrec")
                nc.vector.reciprocal(mrec[:], msum[:])
                k2n = pw.tile([128, 128], bf16, tag="k2n")
                nc.vector.tensor_scalar_mul(k2n[:], e_m[:], mrec[:])
                nsp2 = ps2.tile([128, 4, 128], f32, tag="ns")
                nc.tensor.matmul(nsp2[:, 0, :], k2n[:], ident[:], start=True, stop=True)
                nc.vector.tensor_copy(kmt6[:, p, :], nsp2[:, 0, :])
                nc.vector.tensor_scalar_mul(vm6[:, p, :], nsp2[:, 0, :], gsb[:])
                nc.vector.tensor_scalar_mul(vmt6[:, p, :], k2n[:], gsb[:])

                # stash r denominators
                nc.vector.tensor_copy(s_sb6[:, p, :], ps_s6[:, p, 0:65])
                nc.vector.reciprocal(rrec6[:, p:p + 1], ps_s6[:, p, 64:65])

            def ns_group(g):
                prs = [2 * g, 2 * g + 1]
                dsl = slice(2 * g, 2 * g + 2)
                for it in range(6):
                    e_ps = ps2.tile([128, 4, 128], f32, tag="ns")
                    for k, p in enumerate(prs):
                        nc.tensor.matmul(e_ps[:, k, :], kmt6[:, p, :], vm6[:, p, :],
                                         start=True, stop=True)
                    et_ps = ps2.tile([128, 4, 128], f32, tag="ns")
                    for k, p in enumerate(prs):
                        nc.tensor.matmul(et_ps[:, k, :], vm6[:, p, :], kmt6[:, p, :],
                                         start=True, stop=True)
                    g1 = pn.tile([128, ng, 128], bf16, tag="g1")
                    nc.vector.tensor_tensor(out=g1[:], in0=diags[7][:, dsl, :],
                                            in1=e_ps[:, 0:ng, :], op=SUB)
                    et = pn.tile([128, ng, 128], bf16, tag="et")
                    if act_copies:
                        nc.scalar.activation(et[:], et_ps[:, 0:ng, :], AF.Copy)
                    else:
                        nc.vector.tensor_copy(et[:], et_ps[:, 0:ng, :])
                    p2_ps = ps2.tile([128, 4, 128], f32, tag="ns")
                    for k in range(ng):
                        nc.tensor.matmul(p2_ps[:, k, :], et[:, k, :], g1[:, k, :],
                                         start=True, stop=True)
                    g2 = pn.tile([128, ng, 128], bf16, tag="g2")
                    nc.vector.tensor_tensor(out=g2[:], in0=diags[15][:, dsl, :],
                                            in1=p2_ps[:, 0:ng, :], op=SUB)
                    p3_ps = ps2.tile([128, 4, 128], f32, tag="ns")
                    for k in range(ng):
                        nc.tensor.matmul(p3_ps[:, k, :], et[:, k, :], g2[:, k, :],
                                         start=True, stop=True)
                    g3 = pn.tile([128, ng, 128], bf16, tag="g3")
                    nc.vector.tensor_tensor(out=g3[:], in0=diags[13][:, dsl, :],
                                            in1=p3_ps[:, 0:ng, :], op=SUB)
                    if it < 5:
                        v_ps = ps2.tile([128, 4, 128], f32, tag="ns")
                        for k, p in enumerate(prs):
                            nc.tensor.matmul(v_ps[:, k, :], vmt6[:, p, :], g3[:, k, :],
                                             start=True, stop=True)
                        vt_ps = ps2.tile([128, 4, 128], f32, tag="ns")
                        for k, p in enumerate(prs):
                            nc.tensor.matmul(vt_ps[:, k, :], g3[:, k, :], vmt6[:, p, :],
                                             start=True, stop=True)
                        if act_copies:
                            nc.scalar.activation(vm6[:, dsl, :], v_ps[:, 0:ng, :],
                                                 AF.Copy, scale=0.25)
                            nc.scalar.activation(vmt6[:, dsl, :], vt_ps[:, 0:ng, :],
                                                 AF.Copy, scale=0.25)
                        else:
                            nc.vector.tensor_scalar(vm6[:, dsl, :], v_ps[:, 0:ng, :],
                                                    0.25, scalar2=None, op0=MULT)
                            nc.vector.tensor_scalar(vmt6[:, dsl, :], vt_ps[:, 0:ng, :],
                                                    0.25, scalar2=None, op0=MULT)
                    else:
                        vt_ps = ps2.tile([128, 4, 128], f32, tag="ns")
                        for k, p in enumerate(prs):
                            nc.tensor.matmul(vt_ps[:, k, :], g3[:, k, :], vmt6[:, p, :],
                                             start=True, stop=True)
                        for k, p in enumerate(prs):
                            nc.vector.tensor_scalar(ct6[:, p, :], vt_ps[:, k, :],
                                                    rrec6[:, p:p + 1], scalar2=0.25,
                                                    op0=MULT, op1=MULT)

            def phase_b_all():
                """Flat generator over all pairs; lag-2 queue crosses pairs."""
                xo_tiles = {}

                def do_out(p, j, h, ec, b_sb):
                    if h == 0:
                        xo_new = po.tile([128, 2, 4, 64], bf16, tag="xo")
                        xo_tiles[(p, j)] = xo_new
                    xo = xo_tiles[(p, j)]
                    xp = ps2.tile([128, 4, 128], f32, tag="ns")
                    for k in range(4):
                        nc.tensor.matmul(xp[:, k, 0:65], ec[:, :, k], b_sb[:],
                                         start=True, stop=True)
                    xr = pw.tile([128, 4], f32, tag="xr")
                    nc.vector.reciprocal(xr[:], xp[:, 0:4, 64])
                    nc.vector.tensor_tensor(
                        out=xo[:, h, :, :], in0=xp[:, 0:4, 0:64],
                        in1=xr.rearrange("a (b c) -> a b c", c=1)
                              .to_broadcast([128, 4, 64]),
                        op=MULT)
                    if h == 1:
                        nc.sync.dma_start(
                            XO[p, j * 1024:(j + 1) * 1024, :]
                            .rearrange("(h pp r) d -> pp h (r d)", h=2, pp=128),
                            xo[:].rearrange("a h r d -> a h (r d)"))
                        del xo_tiles[(p, j)]

                pend = []
                for p in range(PAIRS):
                    if p + 2 < PAIRS:
                        load_qt(p + 2)
                    qt_r = qts[p]
                    a_ps = ps2.tile([128, 4, 128], f32, tag="ns")
                    nc.tensor.matmul(a_ps[:, 0, 0:65], ct6[:, p, :], s_sb6[:, p, :],
                                     start=True, stop=True)
                    b_sb = pw.tile([128, 65], bf16, tag="bsb")
                    nc.vector.memset(b_sb[:, 64:65], 1.0)
                    nc.vector.tensor_copy(b_sb[:, 0:64], a_ps[:, 0, 0:64])
                    for j in range(8):
                        for h in range(2):
                            yield
                            n0 = j * 1024 + h * 512
                            if h == 0:
                                cp = ps_share.rearrange("a (b c) -> a b c", c=128)
                            else:
                                cp = ps1.tile([128, 4, 128], f32, tag="c1")
                            nc.tensor.matmul(cp[:].rearrange("a b c -> a (b c)"),
                                             nc6[:, p, :], qt_r[:, n0:n0 + 512],
                                             start=True, stop=True)
                            ec = pw.tile([128, 128, 4], bf16, tag="ec")
                            nc.scalar.activation(
                                ec[:].rearrange("a b c -> a (b c)"),
                                cp[:].rearrange("a b c -> a (b c)"), AF.Exp)
                            pend.append((p, j, h, ec, b_sb))
                            if len(pend) > 2:
                                do_out(*pend.pop(0))
                for it in pend:
                    do_out(*it)

            def drive(primary, aux):
                for _ in primary:
                    if aux is not None:
                        next(aux, None)

            import os
            NSCFG = os.environ.get("NSCFG", "1")
            nsg = None
            for p in range(PAIRS):
                if p < 2:
                    load_qt(p)
                if p == 3:
                    nsg = ns_group([0, 1, 2])
                if p == 5 and NSCFG == "2":
                    for _ in nsg:
                        pass
                    nsg = ns_group([3, 4])
                drive(phase_a(p), nsg if p >= 3 else None)
            if nsg is not None:
                for _ in nsg:
                    pass
            if NSCFG == "2":
                drive(phase_b_all(), ns_group([5]))
            else:
                drive(phase_b_all(), ns_group([3, 4, 5]))
    nc.finalize()
    _cache["nc"] = nc
    return nc


def kernel(Q, K, V, mask):
    from concourse.bass_utils import run_bass_kernel_spmd

    Q = np.asarray(Q, dtype=np.float32)
    K = np.asarray(K, dtype=np.float32)
    V = np.asarray(V, dtype=np.float32)
    BH = B * H
    Qf = Q.reshape(BH, N, D)
    Kf = K.reshape(BH, N, D)
    Vf = V.reshape(BH, N, D)

    nct = np.empty((BH, D, M), np.float32)
    nrt = np.empty((BH, D, M), np.float32)
    gmax = 0.0
    for i in range(BH):
        for (T, out) in ((Kf, nct), (Qf, nrt)):
            s = T[i, :, 0].copy()
            s[0] = np.inf
            idx = np.argpartition(-s, M)[:M]
            out[i] = T[i, np.sort(idx), :].T
        nr = nrt[i].T.astype(np.float64)
        nc_ = nct[i].T.astype(np.float64)
        m = nr @ nc_.T
        e = np.exp(m - m.max(axis=1, keepdims=True))
        k2 = e / e.sum(axis=1, keepdims=True)
        gmax = max(gmax, float(k2.sum(axis=0).max()))

    QTf = np.ascontiguousarray(Qf.transpose(0, 2, 1))
    KTBf = np.ascontiguousarray(Kf.transpose(0, 2, 1)).astype(ml_dtypes.bfloat16)
    NRBf = nrt.astype(ml_dtypes.bfloat16)
    LMf = np.stack([nrt, nct], axis=1)  # [BH, 2, 64, M] f32
    Vb = np.empty((BH, 128, NT, 65), np.float32)
    Vb[:, :, :, 64] = 1.0
    Vb[:, :, :, 0:64] = Vf.reshape(BH, NT, 128, D).transpose(0, 2, 1, 3)
    VBf = Vb.astype(ml_dtypes.bfloat16)
    gs = np.array([[1.0 / gmax]], np.float32)

    nc = _build()
    in_maps = []
    for c in range(NCORES):
        sl = slice(c * PAIRS, (c + 1) * PAIRS)
        in_maps.append({"QT": QTf[sl], "KTB": KTBf[sl], "NRB": NRBf[sl],
                        "LM": LMf[sl], "VB": VBf[sl], "GS": gs})
    res = run_bass_kernel_spmd(nc, in_maps, list(range(NCORES)))
    _cache["last_result"] = res
    X = np.concatenate([res.results[c]["XO"] for c in range(NCORES)], axis=0)
    return X.astype(np.float32).reshape(B, H, N, D)


# revision 44
# speedup vs baseline: 2.8567x; 1.0005x over previous
import numpy as np
import ml_dtypes

B, H, N, D = 4, 12, 8192, 64
M = 128
NCORES = 8
PAIRS = (B * H) // NCORES
NT = N // 128  # 64 column-blocks of 128

_cache = {}


def _build():
    if "nc" in _cache:
        return _cache["nc"]
    import concourse.bacc as bacc
    import concourse.mybir as mybir
    import concourse.tile as tile

    f32, f32r, bf16 = mybir.dt.float32, mybir.dt.float32r, mybir.dt.bfloat16
    AF = mybir.ActivationFunctionType
    MULT = mybir.AluOpType.mult
    SUB = mybir.AluOpType.subtract

    nc = bacc.Bacc("TRN2", target_bir_lowering=False, debug=False)
    QT = nc.declare_dram_parameter("QT", [PAIRS, 64, N], f32, isOutput=False)
    KTB = nc.declare_dram_parameter("KTB", [PAIRS, 64, N], bf16, isOutput=False)
    NRB = nc.declare_dram_parameter("NRB", [PAIRS, 64, M], bf16, isOutput=False)
    LM = nc.declare_dram_parameter("LM", [PAIRS, 2, 64, M], f32, isOutput=False)
    VB = nc.declare_dram_parameter("VB", [PAIRS, 128, NT, 65], bf16, isOutput=False)
    GS = nc.declare_dram_parameter("GS", [1, 1], f32, isOutput=False)
    XO = nc.declare_dram_parameter("XO", [PAIRS, N, 64], bf16, isOutput=True)

    RGROUPS = [4] * 16  # 64 r-blocks in groups (matches rt tile depth 4)

    with tile.TileContext(nc) as tc:
        with (tc.tile_pool(name="pc", bufs=1) as pc,
              tc.tile_pool(name="pio", bufs=2) as pio,
              tc.tile_pool(name="pq", bufs=3) as pq,
              tc.tile_pool(name="pw", bufs=8) as pw,
              tc.tile_pool(name="pn", bufs=4) as pn,
              tc.tile_pool(name="po", bufs=8) as po,
              tc.tile_pool(name="ps1", bufs=1, space="PSUM") as ps1,
              tc.tile_pool(name="psr", bufs=3, space="PSUM") as psr,
              tc.tile_pool(name="ps2", bufs=3, space="PSUM") as ps2):

            # ---- preload pair 0 (DMA starts before const setup) ----
            pre = {}
            pre["ktb"] = pio.tile([64, N], bf16, tag="ktb", name="ktb0")
            pre["nrb"] = pio.tile([64, M], bf16, tag="nrb", name="nrb0")
            pre["vb"] = pio.tile([128, NT, 65], bf16, tag="vb", name="vb0")
            nc.sync.dma_start(pre["nrb"][:], NRB[0])
            for q in range(4):
                nc.sync.dma_start(pre["ktb"][:, q * (N // 4):(q + 1) * (N // 4)],
                                  KTB[0, :, q * (N // 4):(q + 1) * (N // 4)])
            nc.sync.dma_start(pre["vb"][:], VB[0])

            # ---- constants ----
            ident = pc.tile([128, 128], bf16, tag="ident")
            nc.gpsimd.memset(ident[:], 0.0)
            nc.gpsimd.affine_select(out=ident[:], in_=ident[:],
                compare_op=mybir.AluOpType.not_equal, fill=1.0, base=0,
                pattern=[[-1, 128]], channel_multiplier=1)
            diags = {}
            for val in (7, 15, 13):
                t = pc.tile([128, PAIRS, 128], bf16, tag=f"diag{val}")
                nc.gpsimd.memset(t[:], 0.0)
                for p in range(PAIRS):
                    nc.gpsimd.affine_select(out=t[:, p, :], in_=t[:, p, :],
                        compare_op=mybir.AluOpType.not_equal, fill=float(val), base=0,
                        pattern=[[-1, 128]], channel_multiplier=1)
                diags[val] = t
            ones_row = pc.tile([1, 128], f32, tag="ones_row")
            nc.vector.memset(ones_row[:], 1.0)
            gs_sb = pc.tile([1, 1], f32, tag="gs_sb")
            nc.sync.dma_start(gs_sb[:], GS[:])
            nsp0 = ps2.tile([128, 4, 128], f32, tag="ns")
            nc.tensor.matmul(nsp0[:, 0, 0:1], ones_row[:], gs_sb[:], start=True, stop=True)
            gsb = pc.tile([128, 1], f32, tag="gsb")
            nc.vector.tensor_copy(gsb[:], nsp0[:, 0, 0:1])

            # ---- batched NS state ----
            kmt6 = pc.tile([128, PAIRS, 128], bf16, tag="kmt6")
            vm6 = pc.tile([128, PAIRS, 128], bf16, tag="vm6")
            vmt6 = pc.tile([128, PAIRS, 128], bf16, tag="vmt6")
            ct6 = pc.tile([128, PAIRS, 128], bf16, tag="ct6")
            s_sb6 = pc.tile([128, PAIRS, 65], bf16, tag="s_sb6")
            rrec6 = pc.tile([128, PAIRS], f32, tag="rrec6")
            nr6 = pc.tile([64, PAIRS, M], f32r, tag="nr6")
            nc6 = pc.tile([64, PAIRS, M], f32r, tag="nc6")
            ps_share = ps1.tile([128, 512], f32, tag="share")

            qts = {}

            def load_qt(p, defer=False):
                t = pq.tile([64, N], f32r, tag="qt")
                qts[p] = t
                if not defer:
                    for q in range(4):
                        qt_chunk(p, q)

            def qt_chunk(p, q):
                t = qts[p]
                nc.gpsimd.dma_start(t[:, q * (N // 4):(q + 1) * (N // 4)],
                                    QT[p, :, q * (N // 4):(q + 1) * (N // 4)])

            def phase_a(p):
                ktb = pio.tile([64, N], bf16, tag="ktb")
                nrb = pio.tile([64, M], bf16, tag="nrb")
                vb = pio.tile([128, NT, 65], bf16, tag="vb")
                nc.sync.dma_start(ktb[:], KTB[p])
                nc.sync.dma_start(nrb[:], NRB[p])
                nc.gpsimd.dma_start(nr6[:, p, :], LM[p, 0])
                nc.gpsimd.dma_start(nc6[:, p, :], LM[p, 1])
                nc.sync.dma_start(vb[:], VB[p])

                # r-side: r^T blocks -> exp -> S/denominator accumulation
                pending = None
                t0 = 0
                for cnt in RGROUPS:
                    rt = psr.tile([128, 4, 128], f32, tag="rt")
                    for t in range(cnt):
                        nc.tensor.matmul(rt[:, t, :],
                                         ktb[:, (t0 + t) * 128:(t0 + t + 1) * 128],
                                         nrb[:], start=True, stop=True)
                    ert = pw.tile([128, 4, 128], bf16, tag="ert")
                    nc.scalar.activation(ert[:, 0:cnt, :], rt[:, 0:cnt, :], AF.Exp)
                    if pending is not None:
                        pert, pt0, pcnt = pending
                        for t in range(pcnt):
                            nc.tensor.matmul(ps_s6[:, p, 0:65], pert[:, t, :],
                                             vb[:, pt0 + t, :],
                                             start=(pt0 + t == 0), stop=False)
                    pending = (ert, t0, cnt)
                    t0 += cnt
                pert, pt0, pcnt = pending
                for t in range(pcnt):
                    nc.tensor.matmul(ps_s6[:, p, 0:65], pert[:, t, :],
                                     vb[:, pt0 + t, :],
                                     start=False, stop=(pt0 + t == NT - 1))

                # m / k2 / NS init
                nsp = ps2.tile([128, 4, 128], f32, tag="ns")
                nc.tensor.matmul(nsp[:, 0, :], nr6[:, p, :], nc6[:, p, :],
                                 start=True, stop=True)
                e_m = pw.tile([128, 128], bf16, tag="em")
                msum = pw.tile([128, 1], f32, tag="msum")
                nc.scalar.activation(e_m[:], nsp[:, 0, :], AF.Exp, accum_out=msum[:])
                mrec = pw.tile([128, 1], f32, tag="mrec")
                nc.vector.reciprocal(mrec[:], msum[:])
                k2n = pw.tile([128, 128], bf16, tag="k2n")
                nc.vector.tensor_scalar_mul(k2n[:], e_m[:], mrec[:])
                nsp2 = ps2.tile([128, 4, 128], f32, tag="ns")
                nc.tensor.matmul(nsp2[:, 0, :], k2n[:], ident[:], start=True, stop=True)
                nc.vector.tensor_copy(kmt6[:, p, :], nsp2[:, 0, :])
                nc.vector.tensor_scalar_mul(vm6[:, p, :], nsp2[:, 0, :], gsb[:])
                nc.vector.tensor_scalar_mul(vmt6[:, p, :], k2n[:], gsb[:])

                # stash r denominators
                nc.vector.tensor_copy(s_sb6[:, p, :], ps_s6[:, p, 0:65])
                nc.vector.reciprocal(rrec6[:, p:p + 1], ps_s6[:, p, 64:65])

            def ns_group(g):
                prs = [2 * g, 2 * g + 1]
                dsl = slice(2 * g, 2 * g + 2)
                for it in range(6):
                    e_ps = ps2.tile([128, 4, 128], f32, tag="ns")
                    for k, p in enumerate(prs):
                        nc.tensor.matmul(e_ps[:, k, :], kmt6[:, p, :], vm6[:, p, :],
                                         start=True, stop=True)
                    et_ps = ps2.tile([128, 4, 128], f32, tag="ns")
                    for k, p in enumerate(prs):
                        nc.tensor.matmul(et_ps[:, k, :], vm6[:, p, :], kmt6[:, p, :],
                                         start=True, stop=True)
                    g1 = pn.tile([128, ng, 128], bf16, tag="g1")
                    nc.vector.tensor_tensor(out=g1[:], in0=diags[7][:, dsl, :],
                                            in1=e_ps[:, 0:ng, :], op=SUB)
                    et = pn.tile([128, ng, 128], bf16, tag="et")
                    if act_copies:
                        nc.scalar.activation(et[:], et_ps[:, 0:ng, :], AF.Copy)
                    else:
                        nc.vector.tensor_copy(et[:], et_ps[:, 0:ng, :])
                    p2_ps = ps2.tile([128, 4, 128], f32, tag="ns")
                    for k in range(ng):
                        nc.tensor.matmul(p2_ps[:, k, :], et[:, k, :], g1[:, k, :],
                                         start=True, stop=True)
                    g2 = pn.tile([128, ng, 128], bf16, tag="g2")
                    nc.vector.tensor_tensor(out=g2[:], in0=diags[15][:, dsl, :],
                                            in1=p2_ps[:, 0:ng, :], op=SUB)
                    p3_ps = ps2.tile([128, 4, 128], f32, tag="ns")
                    for k in range(ng):
                        nc.tensor.matmul(p3_ps[:, k, :], et[:, k, :], g2[:, k, :],
                                         start=True, stop=True)
                    g3 = pn.tile([128, ng, 128], bf16, tag="g3")
                    nc.vector.tensor_tensor(out=g3[:], in0=diags[13][:, dsl, :],
                                            in1=p3_ps[:, 0:ng, :], op=SUB)
                    if it < 5:
                        v_ps = ps2.tile([128, 4, 128], f32, tag="ns")
                        for k, p in enumerate(prs):
                            nc.tensor.matmul(v_ps[:, k, :], vmt6[:, p, :], g3[:, k, :],
                                             start=True, stop=True)
                        vt_ps = ps2.tile([128, 4, 128], f32, tag="ns")
                        for k, p in enumerate(prs):
                            nc.tensor.matmul(vt_ps[:, k, :], g3[:, k, :], vmt6[:, p, :],
                                             start=True, stop=True)
                        if act_copies:
                            nc.scalar.activation(vm6[:, dsl, :], v_ps[:, 0:ng, :],
                                                 AF.Copy, scale=0.25)
                            nc.scalar.activation(vmt6[:, dsl, :], vt_ps[:, 0:ng, :],
                                                 AF.Copy, scale=0.25)
                        else:
                            nc.vector.tensor_scalar(vm6[:, dsl, :], v_ps[:, 0:ng, :],
                                                    0.25, scalar2=None, op0=MULT)
                            nc.vector.tensor_scalar(vmt6[:, dsl, :], vt_ps[:, 0:ng, :],
                                                    0.25, scalar2=None, op0=MULT)
                    else:
                        vt_ps = ps2.tile([128, 4, 128], f32, tag="ns")
                        for k, p in enumerate(prs):
                            nc.tensor.matmul(vt_ps[:, k, :], g3[:, k, :], vmt6[:, p, :],
                                             start=True, stop=True)
                        for k, p in enumerate(prs):
                            nc.vector.tensor_scalar(ct6[:, p, :], vt_ps[:, k, :],
                                                    rrec6[:, p:p + 1], scalar2=0.25,
                                                    op0=MULT, op1=MULT)

            def phase_b_all():
                """Flat generator over all pairs; lag-2 queue crosses pairs."""
                xo_tiles = {}

                def do_out(p, j, h, ec, b_sb):
                    if h == 0:
                        xo_new = po.tile([128, 2, 4, 64], bf16, tag="xo")
                        xo_tiles[(p, j)] = xo_new
                    xo = xo_tiles[(p, j)]
                    xp = ps2.tile([128, 4, 128], f32, tag="ns")
                    for k in range(4):
                        nc.tensor.matmul(xp[:, k, 0:65], ec[:, :, k], b_sb[:],
                                         start=True, stop=True)
                    xr = pw.tile([128, 4], f32, tag="xr")
                    nc.vector.reciprocal(xr[:], xp[:, 0:4, 64])
                    nc.vector.tensor_tensor(
                        out=xo[:, h, :, :], in0=xp[:, 0:4, 0:64],
                        in1=xr.rearrange("a (b c) -> a b c", c=1)
                              .to_broadcast([128, 4, 64]),
                        op=MULT)
                    if h == 1:
                        nc.sync.dma_start(
                            XO[p, j * 1024:(j + 1) * 1024, :]
                            .rearrange("(h pp r) d -> pp h (r d)", h=2, pp=128),
                            xo[:].rearrange("a h r d -> a h (r d)"))
                        del xo_tiles[(p, j)]

                pend = []
                for p in range(PAIRS):
                    if p + 2 < PAIRS:
                        load_qt(p + 2)
                    qt_r = qts[p]
                    a_ps = ps2.tile([128, 4, 128], f32, tag="ns")
                    nc.tensor.matmul(a_ps[:, 0, 0:65], ct6[:, p, :], s_sb6[:, p, :],
                                     start=True, stop=True)
                    b_sb = pw.tile([128, 65], bf16, tag="bsb")
                    nc.vector.memset(b_sb[:, 64:65], 1.0)
                    nc.vector.tensor_copy(b_sb[:, 0:64], a_ps[:, 0, 0:64])
                    for j in range(8):
                        for h in range(2):
                            yield
                            n0 = j * 1024 + h * 512
                            if h == 0:
                                cp = ps_share.rearrange("a (b c) -> a b c", c=128)
                            else:
                                cp = ps1.tile([128, 4, 128], f32, tag="c1")
                            nc.tensor.matmul(cp[:].rearrange("a b c -> a (b c)"),
                                             nc6[:, p, :], qt_r[:, n0:n0 + 512],
                                             start=True, stop=True)
                            ec = pw.tile([128, 128, 4], bf16, tag="ec")
                            nc.scalar.activation(
                                ec[:].rearrange("a b c -> a (b c)"),
                                cp[:].rearrange("a b c -> a (b c)"), AF.Exp)
                            pend.append((p, j, h, ec, b_sb))
                            if len(pend) > 3:
                                do_out(*pend.pop(0))
                for it in pend:
                    do_out(*it)

            def drive(primary, aux):
                for _ in primary:
                    if aux is not None:
                        next(aux, None)

            import os
            NSCFG = os.environ.get("NSCFG", "1")
            nsg = None
            for p in range(PAIRS):
                if p < 2:
                    load_qt(p)
                if p == 3:
                    nsg = ns_group([0, 1, 2])
                if p == 5 and NSCFG == "2":
                    for _ in nsg:
                        pass
                    nsg = ns_group([3, 4])
                drive(phase_a(p), nsg if p >= 3 else None)
            if nsg is not None:
                for _ in nsg:
                    pass
            if NSCFG == "2":
                drive(phase_b_all(), ns_group([5]))
            else:
                drive(phase_b_all(), ns_group([3, 4, 5]))
    nc.finalize()
    _cache["nc"] = nc
    return nc


def kernel(Q, K, V, mask):
    from concourse.bass_utils import run_bass_kernel_spmd

    Q = np.asarray(Q, dtype=np.float32)
    K = np.asarray(K, dtype=np.float32)
    V = np.asarray(V, dtype=np.float32)
    BH = B * H
    Qf = Q.reshape(BH, N, D)
    Kf = K.reshape(BH, N, D)
    Vf = V.reshape(BH, N, D)

    nct = np.empty((BH, D, M), np.float32)
    nrt = np.empty((BH, D, M), np.float32)
    gmax = 0.0
    for i in range(BH):
        for (T, out) in ((Kf, nct), (Qf, nrt)):
            s = T[i, :, 0].copy()
            s[0] = np.inf
            idx = np.argpartition(-s, M)[:M]
            out[i] = T[i, np.sort(idx), :].T
        nr = nrt[i].T.astype(np.float64)
        nc_ = nct[i].T.astype(np.float64)
        m = nr @ nc_.T
        e = np.exp(m - m.max(axis=1, keepdims=True))
        k2 = e / e.sum(axis=1, keepdims=True)
        gmax = max(gmax, float(k2.sum(axis=0).max()))

    QTf = np.ascontiguousarray(Qf.transpose(0, 2, 1))
    KTBf = np.ascontiguousarray(Kf.transpose(0, 2, 1)).astype(ml_dtypes.bfloat16)
    NRBf = nrt.astype(ml_dtypes.bfloat16)
    LMf = np.stack([nrt, nct], axis=1)  # [BH, 2, 64, M] f32
    Vb = np.empty((BH, 128, NT, 65), np.float32)
    Vb[:, :, :, 64] = 1.0
    Vb[:, :, :, 0:64] = Vf.reshape(BH, NT, 128, D).transpose(0, 2, 1, 3)
    VBf = Vb.astype(ml_dtypes.bfloat16)
    gs = np.array([[1.0 / gmax]], np.float32)

    nc = _build()
    in_maps = []
    for c in range(NCORES):
        sl = slice(c * PAIRS, (c + 1) * PAIRS)
        in_maps.append({"QT": QTf[sl], "KTB": KTBf[sl], "NRB": NRBf[sl],
                        "LM": LMf[sl], "VB": VBf[sl], "GS": gs})
    res = run_bass_kernel_spmd(nc, in_maps, list(range(NCORES)))
    _cache["last_result"] = res
    X = np.concatenate([res.results[c]["XO"] for c in range(NCORES)], axis=0)
    return X.astype(np.float32).reshape(B, H, N, D)
